# revision 23
# baseline (speedup 1.0000x reference)
"""Trainium2 Bass kernel for BasedLinearAttention (Taylor-feature linear attention).

Full inputs -> full output. Sharding: data-parallel over batch (2) x
tensor-parallel over heads (12 heads / 4 cores = 3 heads/core); 8 cores total.
Each core computes its 3 heads' attention + the partial output projection;
the host sums the 4 per-core partials of each batch (TP row-parallel reduce).

Math notes:
  phi(x) = [1, x/d^(1/4), vec(x (x) x)/(sqrt2 sqrt d)], d=16, D=273.
  phi(k).phi(q) = 1 + u/sqrt(d) + u^2/(2d),  u = k.q
               = (u/sqrt(2d) + 1/sqrt(2))^2 + 0.5
  so intra-chunk scores never materialize phi. The result is chunk-size
  invariant; we use CHUNK=256 (reference uses 64).
  State feature order: [q(x)q (256) | x (16) | 1] with the
  s^2 = [1/(2d)=1/32, 1/sqrt(d)=1/4, 1] scaling folded into the K-side
  features (exact powers of two).
All matmuls run in bf16 (f32 PSUM accumulation): full PE rate at any free
size, no fp32-mode power throttle. Verified end-to-end max-rel ~4e-3 vs
the f32 reference (gate 2e-2).

q^T/k^T come from a transposed projection pass (W stationary, x^T moving)
instead of per-tile PE transposes; per-head tiles are carved out of the
packed PSUM result with SBUF->SBUF shift DMAs (engines can't cross
partitions, DMA can).

Intra-chunk causal structure: with chunk 256 = 2x128 seq tiles, only the
two diagonal 128x128 blocks need masking, the lower block is all-ones and
the upper block is identically zero and never computed. Scores for one
chunk live in a [128, 384] layout = [m0 x (c0|c1) | m1 x c1].
"""

import sys
import zlib
import numpy as np
import ml_dtypes
from contextlib import ExitStack

sys.path.insert(0, "/opt/trn_rl_repo")
sys.path.insert(0, "/opt/trn_rl_repo/pypackages")

import concourse.bass as bass
import concourse.tile as tile
from concourse import bacc
from concourse import mybir
from concourse.bass_utils import run_bass_kernel_spmd

F32 = mybir.dt.float32
BF16 = mybir.dt.bfloat16
ALU = mybir.AluOpType
AF = mybir.ActivationFunctionType
BF_NP = ml_dtypes.bfloat16

B, L, H = 2, 2048, 1536
NH, FD, HD = 12, 16, 128
HPC = 3            # heads per core
CH = 256           # chunk length
NCH = L // CH      # 8 chunks
NKT = H // 128     # 12 contraction tiles for projections
NST = L // 128     # 16 seq tiles
PH = HD + 1            # 129: [v | ones-slot] per head
W3 = HPC * PH + 1      # 388 (padded even)
WT = HPC * 2 * FD      # 96: [k0 q0 k1 q1 k2 q2] transposed-projection cols

_SQ_SCALE = 1.0 / np.sqrt(2.0 * FD)   # 1/sqrt(32)
_SQ_BIAS = 1.0 / np.sqrt(2.0)

# The neuron NEFF cache can false-hit across different BIR with identical
# HLO I/O shapes; encode (source crc, reps) into a dummy input's shape.
try:
    _SRC_CRC = zlib.crc32(open(__file__, "rb").read()) % 1024
except OSError:
    _SRC_CRC = 0


def _bust_shape(reps):
    return [reps, 8 + _SRC_CRC]


def _fill(nc, ap, val):
    # real memset: the Identity(in*0 + bias) ACT trick reads uninitialized
    # SBUF, and 0 * NaN = NaN on a cold core. DVE, not gpsimd: any gpsimd
    # activity drops the PE utilization cap to 4/8 for its whole window.
    nc.vector.memset(ap, float(val))


def _build_nc(reps=1):
    nc = bacc.Bacc("TRN2", target_bir_lowering=False, debug=False)
    xt = nc.declare_dram_parameter("xt", [H, L], BF16, isOutput=False)
    wqkv = nc.declare_dram_parameter("wqkv", [H, W3], BF16, isOutput=False)
    wqkt = nc.declare_dram_parameter("wqkt", [H, WT], BF16, isOutput=False)
    wot = nc.declare_dram_parameter("wot", [HPC * HD, H], BF16, isOutput=False)
    maskt = nc.declare_dram_parameter("maskt", [128, 384], BF16, isOutput=False)
    ident = nc.declare_dram_parameter("ident", [128, 128], BF16, isOutput=False)
    onesrow = nc.declare_dram_parameter("onesrow", [1, L], BF16, isOutput=False)
    out = nc.declare_dram_parameter("out", [L, H], BF16, isOutput=True)
    nc.declare_dram_parameter("cachebust", _bust_shape(reps), F32, isOutput=False)

    with tile.TileContext(nc) as tc, ExitStack() as ctx:
        const = ctx.enter_context(tc.tile_pool(name="const", bufs=1))
        wqkt_s = const.tile([128, NKT, WT], BF16)
        nc.sync.dma_start(wqkt_s[:], wqkt.rearrange("(k p) n -> p k n", p=128))
        wq_s = const.tile([128, NKT, W3], BF16)
        nc.scalar.dma_start(wq_s[:], wqkv.rearrange("(k p) n -> p k n", p=128))
        xt_t = const.tile([128, NKT, L], BF16, name="xt_t")
        for quarter in range(4):
            qsl = slice(quarter * 512, (quarter + 1) * 512)
            for kt in range(NKT):
                eng = nc.sync if kt % 2 == 0 else nc.scalar
                eng.dma_start(xt_t[:, kt, qsl], xt[kt * 128:(kt + 1) * 128, qsl])
        ident_s = const.tile([128, 128], BF16)
        nc.sync.dma_start(ident_s[:], ident[:])
        mask_s = const.tile([128, 384], BF16)
        nc.sync.dma_start(mask_s[:], maskt[:])
        ones_s = const.tile([128, 1], BF16)
        _fill(nc, ones_s[:], 1.0)
        onesrow_s = const.tile([1, 128], BF16)
        _fill(nc, onesrow_s[:], 1.0)
        sqb_s = const.tile([128, 1], F32)
        nc.vector.memset(sqb_s[:], _SQ_BIAS)
        wo_s = const.tile([128, HPC, H], BF16)
        nc.scalar.dma_start(wo_s[:], wot.rearrange("(h p) n -> p h n", p=128))
        qkv_s = const.tile([128, NST, W3], BF16)
        onorm = const.tile([128, HPC, L], BF16)
        # per-head transposed tiles: k^T [16, L], [q;1]^T [17, L]
        kt_h = [const.tile([16, L], BF16, name=f"kt{h}") for h in range(HPC)]
        qot_h = [const.tile([17, L], BF16, name=f"qot{h}") for h in range(HPC)]
        for h in range(HPC):
            nc.sync.dma_start(qot_h[h][16:17, :], onesrow[:])

        for rep in range(reps):
            ctx_r = ExitStack()
            hp = ctx_r.enter_context(tc.tile_pool(name=f"hd{rep}", bufs=1))
            phiqt_h = [hp.tile([128, 2, L], BF16, name=f"phiqt{h}")
                       for h in range(HPC)]
            phik_h = [hp.tile([128, NST, 273], BF16, name=f"phik{h}")
                      for h in range(HPC)]
            qk_h = [hp.tile([128, NST, 32], BF16, name=f"qksm{h}")
                    for h in range(HPC)]
            # ---- Stage 1: projections fused with phi-feature building ----
            with (
                tc.tile_pool(name=f"qkvps{rep}", bufs=4, space="PSUM") as qps,
                tc.tile_pool(name=f"qktps{rep}", bufs=2, space="PSUM") as tps,
                tc.tile_pool(name=f"tp{rep}", bufs=2, space="PSUM") as tp,
                tc.tile_pool(name=f"qkstg{rep}", bufs=2) as stg,
                tc.tile_pool(name=f"p2{rep}", bufs=2) as p2,
            ):
                for h in range(HPC):
                    _fill(nc, phik_h[h][:, :, 272:273], 1.0)

                for quarter in range(4):
                    qsl = slice(quarter * 512, (quarter + 1) * 512)
                    # transposed q/k projection: out rows [k0 q0 k1 q1 k2 q2]
                    pt = tps.tile([WT, 512], F32, tag="pt")
                    for kt in range(NKT):
                        nc.tensor.matmul(
                            pt[:], wqkt_s[:, kt, :], xt_t[:, kt, qsl],
                            start=(kt == 0), stop=(kt == NKT - 1),
                        )
                    qkstg = stg.tile([WT, 512], BF16, tag="stg")
                    if quarter % 2 == 0:
                        nc.vector.tensor_copy(qkstg[:], pt[:])
                    else:
                        nc.scalar.copy(qkstg[:], pt[:])
                    for h in range(HPC):
                        nc.scalar.dma_start(
                            kt_h[h][:, qsl], qkstg[32 * h:32 * h + 16, :])
                        nc.scalar.dma_start(
                            qot_h[h][0:16, qsl], qkstg[32 * h + 16:32 * h + 32, :])
                    # seq-major q/k recovered from the transposed tiles by DMA
                    for s4 in range(4):
                        s = quarter * 4 + s4
                        sl = slice(s * 128, (s + 1) * 128)
                        for h in range(HPC):
                            eng = nc.sync if (s + h) % 2 == 0 else nc.scalar
                            eng.dma_start_transpose(
                                qk_h[h][:, s, 0:16], qot_h[h][0:16, sl])
                            eng.dma_start_transpose(
                                qk_h[h][:, s, 16:32], kt_h[h][:, sl])
                    # seq-major v projection + phi features per seq tile
                    for s4 in range(4):
                        s = quarter * 4 + s4
                        sl = slice(s * 128, (s + 1) * 128)
                        ps = qps.tile([128, W3], F32, tag="ps")
                        for kt in range(NKT):
                            nc.tensor.matmul(
                                ps[:],
                                xt_t[:, kt, sl],
                                wq_s[:, kt, :],
                                start=(kt == 0),
                                stop=(kt == NKT - 1),
                            )
                        if s % 4 == 0:
                            nc.vector.tensor_copy(qkv_s[:, s, :], ps[:])
                        else:
                            nc.scalar.copy(qkv_s[:, s, :], ps[:])
                        for h in range(HPC):
                            phiqt = phiqt_h[h]
                            phik = phik_h[h]
                            qsl_ = qk_h[h][:, s, 0:16]
                            p2n = p2.tile([128, 256], BF16, tag="p2n")
                            nc.gpsimd.tensor_tensor(
                                p2n[:].rearrange("p (a b) -> p a b", a=16),
                                qsl_.unsqueeze(-1).broadcast_to([128, 16, 16]),
                                qsl_.unsqueeze(1).broadcast_to([128, 16, 16]),
                                op=ALU.mult,
                            )
                            p2t = tp.tile([128, 256], BF16, tag="p2t")
                            nc.tensor.transpose(
                                p2t[:, 0:128], p2n[:, 0:128], ident_s[:])
                            nc.tensor.transpose(
                                p2t[:, 128:256], p2n[:, 128:256], ident_s[:])
                            if (s + h) % 2 == 0:
                                nc.vector.tensor_copy(
                                    phiqt[:, :, sl],
                                    p2t[:].rearrange("p (t c) -> p t c", t=2),
                                )
                            else:
                                nc.scalar.copy(
                                    phiqt[:, :, sl],
                                    p2t[:].rearrange("p (t c) -> p t c", t=2),
                                )
                            ksl = qk_h[h][:, s, 16:32]
                            nc.vector.scalar_tensor_tensor(
                                phik[:, s, 0:256].rearrange("p (a b) -> p a b", a=16),
                                ksl.unsqueeze(-1).broadcast_to([128, 16, 16]),
                                1.0 / 32.0,
                                ksl.unsqueeze(1).broadcast_to([128, 16, 16]),
                                op0=ALU.mult,
                                op1=ALU.mult,
                            )
                            nc.gpsimd.tensor_scalar_mul(
                                phik[:, s, 256:272], ksl, 0.25)

            # ones-slot column of each head's v-ext block (after the stage-1
            # copies, which write projection zeros there)
            for h in range(HPC):
                _fill(nc, qkv_s[:, :, h * PH + 128:h * PH + 129], 1.0)

            # ---- Stage 2: chunked scan, heads interleaved per chunk, with
            # ---- the output projection (stage 3) folded in per chunk
            with (
                tc.tile_pool(name=f"u{rep}", bufs=1, space="PSUM") as up,
                tc.tile_pool(name=f"pz{rep}", bufs=3, space="PSUM") as pzp,
                tc.tile_pool(name=f"kv{rep}", bufs=1, space="PSUM") as kvp,
                tc.tile_pool(name=f"st{rep}", bufs=2) as stp,
                tc.tile_pool(name=f"ost{rep}", bufs=2) as osp,
            ):
                ksizes = (128, 128, 17)
                koffs = (0, 128, 256)
                # per-head state: [kvA(132) | kvB(132) | kvC rows 0:17 (132)]
                kvm_h = [kvp.tile([128, 396], F32, name=f"kvm{h}")
                         for h in range(HPC)]
                kvt_h = [(kvm_h[h][:, 0:132], kvm_h[h][:, 132:264],
                          kvm_h[h][0:17, 264:396]) for h in range(HPC)]
                # score PSUM, manual ping-pong pair (2 banks)
                put = up.tile([128, 2, 384], F32, name="put")
                snap_h = [None] * HPC
                pu_h = [None] * HPC
                stm_h = [None] * HPC
                for n in range(NCH):
                    cs = slice(n * CH, (n + 1) * CH)
                    cs2 = slice(n * CH + 128, (n + 1) * CH)
                    ms0 = slice(n * CH, n * CH + 128)
                    ms1 = slice(n * CH + 128, (n + 1) * CH)
                    for h in range(HPC):
                        # u[m, c] = k_m . q_c ; layout [m0 x (c0|c1) | m1 x c1]
                        pu = put[:, (n * HPC + h) % 2, :]
                        nc.tensor.matmul(
                            pu[:, 0:256], kt_h[h][:, ms0], qot_h[h][0:16, cs],
                            start=True, stop=True, skip_group_check=True,
                        )
                        nc.tensor.matmul(
                            pu[:, 256:384], kt_h[h][:, ms1], qot_h[h][0:16, cs2],
                            start=True, stop=True, skip_group_check=True,
                        )
                        # st = (u/sqrt(32) + 1/sqrt(2))^2 + 0.5, causal-masked
                        straw = stp.tile([128, 384], F32, tag="straw")
                        nc.scalar.activation(
                            straw[:], pu[:], AF.Square,
                            bias=sqb_s[:], scale=_SQ_SCALE,
                        )
                        stm = stp.tile([128, 384], BF16, tag="stm",
                                       name=f"stm{rep}_{h}_{n}")
                        nc.vector.scalar_tensor_tensor(
                            stm[:], straw[:], 0.5, mask_s[:],
                            op0=ALU.add, op1=ALU.mult,
                        )
                        pu_h[h] = pu
                        stm_h[h] = stm
                    for h in range(HPC):
                        voff = h * PH
                        phiqt = phiqt_h[h]
                        phik = phik_h[h]
                        stm = stm_h[h]
                        snap = snap_h[h]
                        # merged [po (cols 0:256) | z row (cols 256:512)]
                        pz = pzp.tile([128, 512], F32, tag="pz",
                                      name=f"pz{rep}_{h}_{n}")
                        po = pz[:, 0:256]
                        zz = pz[0:1, 256:512]
                        nmm = 2 if n == 0 else 5
                        # z[c] = sum_m st[m,c] + phiQ[c] . ks
                        nc.tensor.matmul(
                            zz[0:1, 0:256], ones_s[:], stm[:, 0:256],
                            start=True, stop=(nmm == 2),
                        )
                        nc.tensor.matmul(
                            zz[0:1, 128:256], ones_s[:], stm[:, 256:384],
                            start=False, stop=(n == 0), skip_group_check=True,
                        )
                        if n > 0:
                            for t in range(3):
                                kd = ksizes[t]
                                rhs = (phiqt[0:128, t, cs] if t < 2
                                       else qot_h[h][0:17, cs])
                                nc.tensor.matmul(
                                    zz[0:1, 0:256], snap[t][0:kd, 128:129], rhs,
                                    start=False, stop=(t == 2),
                                    skip_group_check=True,
                                )
                        # 1/z (fp32 fast approx), broadcast to partitions
                        zr = stp.tile([1, CH], F32, tag="zr")
                        nc.vector.reciprocal_approx_fast(zr[:], zz[0:1, :])
                        zrb = stp.tile([128, CH], F32, tag="zrb",
                                       name=f"zrb{rep}_{h}_{n}")
                        nc.gpsimd.partition_broadcast(zrb[:], zr[0:1, :])
                        # o^T[d, c] = sum_m v[m,d] st[m,c] + sum_D kv[D,d] phiQ^T[D,c]
                        oi = 2
                        nc.tensor.matmul(
                            po[:, 0:256], qkv_s[:, 2 * n, voff:voff + 128],
                            stm[:, 0:256], start=True, stop=(nmm == 2),
                        )
                        nc.tensor.matmul(
                            po[:, 128:256], qkv_s[:, 2 * n + 1, voff:voff + 128],
                            stm[:, 256:384], start=False, stop=(oi == nmm),
                            skip_group_check=True,
                        )
                        if n > 0:
                            for t in range(3):
                                kd = ksizes[t]
                                rhs = (phiqt[0:128, t, cs] if t < 2
                                       else qot_h[h][0:17, cs])
                                oi += 1
                                nc.tensor.matmul(
                                    po[:, 0:256], snap[t][0:kd, 0:128], rhs,
                                    start=False, stop=(oi == nmm),
                                    skip_group_check=True,
                                )
                        # normalized, transposed output slice
                        nc.vector.tensor_tensor(
                            onorm[:, h, cs], po[:, :], zrb[:], op=ALU.mult,
                        )
                        # state += phiK_chunk^T @ [v | 1]
                        kvt = kvt_h[h]
                        for mt in range(2):
                            s = 2 * n + mt
                            for t in range(3):
                                kd = ksizes[t]
                                co = koffs[t]
                                nc.tensor.matmul(
                                    kvt[t][0:kd, 0:129],
                                    phik[:, s, co:co + kd],
                                    qkv_s[:, s, voff:voff + 129],
                                    start=(n == 0 and mt == 0),
                                    stop=(n == NCH - 1 and mt == 1),
                                )
                        if n < NCH - 1:
                            sA = stp.tile([128, 132], BF16, tag=f"snapA{h}",
                                          name=f"snA{rep}_{h}_{n}")
                            sB = stp.tile([128, 132], BF16, tag=f"snapB{h}",
                                          name=f"snB{rep}_{h}_{n}")
                            sC = stp.tile([17, 132], BF16, tag=f"snapC{h}",
                                          name=f"snC{rep}_{h}_{n}")
                            nc.vector.tensor_copy(sA[:, 0:129], kvt[0][:, 0:129])
                            nc.scalar.copy(sB[:, 0:129], kvt[1][:, 0:129])
                            nc.scalar.copy(sC[0:17, 0:129], kvt[2][0:17, 0:129])
                            snap_h[h] = (sA, sB, sC)
                    # output projection for this chunk's two seq tiles
                    for s in (2 * n, 2 * n + 1):
                        sl = slice(s * 128, (s + 1) * 128)
                        ob = osp.tile([128, H], BF16, tag="ob",
                                      name=f"ob{rep}_{s}")
                        for j in range(3):
                            pso = pzp.tile([128, 512], F32, tag="pz",
                                           name=f"pso{rep}_{s}_{j}")
                            for h in range(HPC):
                                nc.tensor.matmul(
                                    pso[:],
                                    onorm[:, h, sl],
                                    wo_s[:, h, j * 512:(j + 1) * 512],
                                    start=(h == 0),
                                    stop=(h == HPC - 1),
                                )
                            if j == 0:
                                nc.vector.tensor_copy(
                                    ob[:, j * 512:(j + 1) * 512], pso[:])
                            else:
                                nc.scalar.copy(
                                    ob[:, j * 512:(j + 1) * 512], pso[:])
                        eng = nc.sync if s % 2 == 0 else nc.scalar
                        eng.dma_start(out[sl, :], ob[:])
            ctx_r.close()

    nc.compile()
    return nc


_NC_CACHE = None


def _get_nc():
    global _NC_CACHE
    if _NC_CACHE is None:
        _NC_CACHE = _build_nc()
    return _NC_CACHE


def _in_maps(hidden_states, Wq, Wk, Wv, Wo, reps=1):
    ut = (np.arange(128)[:, None] <= np.arange(128)[None, :]).astype(np.float32)
    maskt = np.concatenate([ut, np.ones((128, 128), np.float32), ut], axis=1)
    ident = np.eye(128, dtype=np.float32)
    maps = []
    for c in range(8):
        b, hg = c // 4, c % 4
        heads = [hg * HPC + j for j in range(HPC)]
        xtb = np.ascontiguousarray(hidden_states[b].T)
        wqkv = np.zeros((H, W3), np.float32)
        wqkt = np.zeros((H, WT), np.float32)
        wot = np.empty((HPC * HD, H), np.float32)
        for j, hh in enumerate(heads):
            o = j * PH
            wqkv[:, o:o + HD] = Wv[hh * HD:(hh + 1) * HD].T
            # o + 128 is the v-ext ones-slot (zero weights)
            wqkt[:, 32 * j:32 * j + 16] = Wk[hh * FD:(hh + 1) * FD].T
            wqkt[:, 32 * j + 16:32 * j + 32] = Wq[hh * FD:(hh + 1) * FD].T
            wot[j * HD:(j + 1) * HD, :] = Wo[:, hh * HD:(hh + 1) * HD].T
        maps.append({
            "xt": xtb.astype(BF_NP),
            "wqkv": wqkv.astype(BF_NP),
            "wqkt": wqkt.astype(BF_NP),
            "wot": wot.astype(BF_NP),
            "maskt": maskt.astype(BF_NP),
            "ident": ident.astype(BF_NP),
            "onesrow": np.ones((1, L), BF_NP),
            "cachebust": np.zeros(_bust_shape(reps), np.float32),
        })
    return maps


def kernel(hidden_states, Wq, Wk, Wv, Wo):
    nc = _get_nc()
    maps = _in_maps(
        np.asarray(hidden_states, np.float32), np.asarray(Wq, np.float32),
        np.asarray(Wk, np.float32), np.asarray(Wv, np.float32),
        np.asarray(Wo, np.float32),
    )
    res = run_bass_kernel_spmd(nc, maps, core_ids=list(range(8)))
    out = np.zeros((B, L, H), np.float32)
    for c in range(8):
        out[c // 4] += res.results[c]["out"].astype(np.float32)
    return out


# revision 24
# speedup vs baseline: 1.2505x; 1.2505x over previous
"""Trainium2 Bass kernel for BasedLinearAttention (Taylor-feature linear attention).

Full inputs -> full output. Sharding: data-parallel over batch (2) x
tensor-parallel over heads (12 heads / 4 cores = 3 heads/core); 8 cores total.
Each core computes its 3 heads' attention + the partial output projection;
the host sums the 4 per-core partials of each batch (TP row-parallel reduce).

Math notes:
  phi(x) = [1, x/d^(1/4), vec(x (x) x)/(sqrt2 sqrt d)], d=16, D=273.
  phi(k).phi(q) = 1 + u/sqrt(d) + u^2/(2d),  u = k.q
               = (u/sqrt(2d) + 1/sqrt(2))^2 + 0.5
  so intra-chunk scores never materialize phi. The result is chunk-size
  invariant; we use CHUNK=256 (reference uses 64).
  State feature order: [q(x)q (256) | x (16) | 1] with the
  s^2 = [1/(2d)=1/32, 1/sqrt(d)=1/4, 1] scaling folded into the K-side
  features (exact powers of two).
All matmuls run in bf16 (f32 PSUM accumulation): full PE rate at any free
size, no fp32-mode power throttle. Verified end-to-end max-rel ~4e-3 vs
the f32 reference (gate 2e-2).

q^T/k^T come from a transposed projection pass (W stationary, x^T moving)
instead of per-tile PE transposes; per-head tiles are carved out of the
packed PSUM result with SBUF->SBUF shift DMAs (engines can't cross
partitions, DMA can).

Intra-chunk causal structure: with chunk 256 = 2x128 seq tiles, only the
two diagonal 128x128 blocks need masking, the lower block is all-ones and
the upper block is identically zero and never computed. Scores for one
chunk live in a [128, 384] layout = [m0 x (c0|c1) | m1 x c1].
"""

import sys
import zlib
import numpy as np
import ml_dtypes
from contextlib import ExitStack

sys.path.insert(0, "/opt/trn_rl_repo")
sys.path.insert(0, "/opt/trn_rl_repo/pypackages")

import concourse.bass as bass
import concourse.tile as tile
from concourse import bacc
from concourse import mybir
from concourse.bass_utils import run_bass_kernel_spmd

F32 = mybir.dt.float32
BF16 = mybir.dt.bfloat16
ALU = mybir.AluOpType
AF = mybir.ActivationFunctionType
BF_NP = ml_dtypes.bfloat16

B, L, H = 2, 2048, 1536
NH, FD, HD = 12, 16, 128
HPC = 3            # heads per core
CH = 256           # chunk length
NCH = L // CH      # 8 chunks
NKT = H // 128     # 12 contraction tiles for projections
NST = L // 128     # 16 seq tiles
PH = 2 * FD + HD + 1   # 161: [q | k | v | ones-slot] per head
W3 = HPC * PH + 1      # 484 (padded even)
WT = HPC * 2 * FD      # 96: [k0 q0 k1 q1 k2 q2] transposed-projection cols

_SQ_SCALE = 1.0 / np.sqrt(2.0 * FD)   # 1/sqrt(32)
_SQ_BIAS = 1.0 / np.sqrt(2.0)

# The neuron NEFF cache can false-hit across different BIR with identical
# HLO I/O shapes; encode (source crc, reps) into a dummy input's shape.
try:
    _SRC_CRC = zlib.crc32(open(__file__, "rb").read()) % 1024
except OSError:
    _SRC_CRC = 0


def _bust_shape(reps):
    return [reps, 8 + _SRC_CRC]


def _fill(nc, ap, val):
    # real memset: the Identity(in*0 + bias) ACT trick reads uninitialized
    # SBUF, and 0 * NaN = NaN on a cold core. DVE, not gpsimd: any gpsimd
    # activity drops the PE utilization cap to 4/8 for its whole window.
    nc.vector.memset(ap, float(val))


def _build_nc(reps=1):
    nc = bacc.Bacc("TRN2", target_bir_lowering=False, debug=False)
    xt = nc.declare_dram_parameter("xt", [H, L], BF16, isOutput=False)
    wqkv = nc.declare_dram_parameter("wqkv", [H, W3], BF16, isOutput=False)
    wqkt = nc.declare_dram_parameter("wqkt", [H, WT], BF16, isOutput=False)
    wot = nc.declare_dram_parameter("wot", [HPC * HD, H], BF16, isOutput=False)
    maskt = nc.declare_dram_parameter("maskt", [128, 384], BF16, isOutput=False)
    ident = nc.declare_dram_parameter("ident", [128, 128], BF16, isOutput=False)
    onesrow = nc.declare_dram_parameter("onesrow", [1, L], BF16, isOutput=False)
    out = nc.declare_dram_parameter("out", [L, H], BF16, isOutput=True)
    nc.declare_dram_parameter("cachebust", _bust_shape(reps), F32, isOutput=False)

    with tile.TileContext(nc) as tc, ExitStack() as ctx:
        const = ctx.enter_context(tc.tile_pool(name="const", bufs=1))
        wqkt_s = const.tile([128, NKT, WT], BF16)
        nc.sync.dma_start(wqkt_s[:], wqkt.rearrange("(k p) n -> p k n", p=128))
        wq_s = const.tile([128, NKT, W3], BF16)
        nc.scalar.dma_start(wq_s[:], wqkv.rearrange("(k p) n -> p k n", p=128))
        xt_t = const.tile([128, NKT, L], BF16, name="xt_t")
        for quarter in range(4):
            qsl = slice(quarter * 512, (quarter + 1) * 512)
            for kt in range(NKT):
                eng = nc.sync if kt % 2 == 0 else nc.scalar
                eng.dma_start(xt_t[:, kt, qsl], xt[kt * 128:(kt + 1) * 128, qsl])
        ident_s = const.tile([128, 128], BF16)
        nc.sync.dma_start(ident_s[:], ident[:])
        mask_s = const.tile([128, 384], BF16)
        nc.sync.dma_start(mask_s[:], maskt[:])
        ones_s = const.tile([128, 1], BF16)
        _fill(nc, ones_s[:], 1.0)
        onesrow_s = const.tile([1, 128], BF16)
        _fill(nc, onesrow_s[:], 1.0)
        sqb_s = const.tile([128, 1], F32)
        nc.vector.memset(sqb_s[:], _SQ_BIAS)
        wo_s = const.tile([128, HPC, H], BF16)
        nc.scalar.dma_start(wo_s[:], wot.rearrange("(h p) n -> p h n", p=128))
        qkv_s = const.tile([128, NST, W3], BF16)
        onorm = const.tile([128, HPC, L], BF16)
        # per-head transposed tiles: k^T [16, L], [q;1]^T [17, L]
        kt_h = [const.tile([16, L], BF16, name=f"kt{h}") for h in range(HPC)]
        qot_h = [const.tile([17, L], BF16, name=f"qot{h}") for h in range(HPC)]
        for h in range(HPC):
            nc.sync.dma_start(qot_h[h][16:17, :], onesrow[:])

        for rep in range(reps):
            ctx_r = ExitStack()
            hp = ctx_r.enter_context(tc.tile_pool(name=f"hd{rep}", bufs=1))
            phiqt_h = [hp.tile([128, 2, L], BF16, name=f"phiqt{h}")
                       for h in range(HPC)]
            phik_h = [hp.tile([128, NST, 273], BF16, name=f"phik{h}")
                      for h in range(HPC)]

            # ---- Stage 1: projections fused with phi-feature building ----
            with (
                tc.tile_pool(name=f"qkvps{rep}", bufs=4, space="PSUM") as qps,
                tc.tile_pool(name=f"qktps{rep}", bufs=2, space="PSUM") as tps,
                tc.tile_pool(name=f"tp{rep}", bufs=2, space="PSUM") as tp,
                tc.tile_pool(name=f"qkstg{rep}", bufs=2) as stg,
                tc.tile_pool(name=f"p2{rep}", bufs=2) as p2,
            ):
                for h in range(HPC):
                    _fill(nc, phik_h[h][:, :, 272:273], 1.0)

                for quarter in range(4):
                    qsl = slice(quarter * 512, (quarter + 1) * 512)
                    # transposed q/k projection: out rows [k0 q0 k1 q1 k2 q2]
                    pt = tps.tile([WT, 512], F32, tag="pt")
                    for kt in range(NKT):
                        nc.tensor.matmul(
                            pt[:], wqkt_s[:, kt, :], xt_t[:, kt, qsl],
                            start=(kt == 0), stop=(kt == NKT - 1),
                        )
                    qkstg = stg.tile([WT, 512], BF16, tag="stg")
                    if quarter % 2 == 0:
                        nc.vector.tensor_copy(qkstg[:], pt[:])
                    else:
                        nc.scalar.copy(qkstg[:], pt[:])
                    for h in range(HPC):
                        nc.scalar.dma_start(
                            kt_h[h][:, qsl], qkstg[32 * h:32 * h + 16, :])
                        nc.scalar.dma_start(
                            qot_h[h][0:16, qsl], qkstg[32 * h + 16:32 * h + 32, :])
                    # seq-major q/k/v projection + phi features per seq tile
                    for s4 in range(4):
                        s = quarter * 4 + s4
                        sl = slice(s * 128, (s + 1) * 128)
                        ps = qps.tile([128, W3], F32, tag="ps")
                        for kt in range(NKT):
                            nc.tensor.matmul(
                                ps[:],
                                xt_t[:, kt, sl],
                                wq_s[:, kt, :],
                                start=(kt == 0),
                                stop=(kt == NKT - 1),
                            )
                        if s % 4 == 0:
                            nc.vector.tensor_copy(qkv_s[:, s, :], ps[:])
                        else:
                            nc.scalar.copy(qkv_s[:, s, :], ps[:])
                        for h in range(HPC):
                            qoff = h * PH
                            phiqt = phiqt_h[h]
                            phik = phik_h[h]
                            qsl_ = qkv_s[:, s, qoff:qoff + 16]
                            p2n = p2.tile([128, 256], BF16, tag="p2n")
                            nc.gpsimd.tensor_tensor(
                                p2n[:].rearrange("p (a b) -> p a b", a=16),
                                qsl_.unsqueeze(-1).broadcast_to([128, 16, 16]),
                                qsl_.unsqueeze(1).broadcast_to([128, 16, 16]),
                                op=ALU.mult,
                            )
                            p2t = tp.tile([128, 256], BF16, tag="p2t")
                            nc.tensor.transpose(
                                p2t[:, 0:128], p2n[:, 0:128], ident_s[:])
                            nc.tensor.transpose(
                                p2t[:, 128:256], p2n[:, 128:256], ident_s[:])
                            if (s + h) % 2 == 0:
                                nc.vector.tensor_copy(
                                    phiqt[:, :, sl],
                                    p2t[:].rearrange("p (t c) -> p t c", t=2),
                                )
                            else:
                                nc.scalar.copy(
                                    phiqt[:, :, sl],
                                    p2t[:].rearrange("p (t c) -> p t c", t=2),
                                )
                            ksl = qkv_s[:, s, qoff + FD:qoff + 2 * FD]
                            nc.vector.scalar_tensor_tensor(
                                phik[:, s, 0:256].rearrange("p (a b) -> p a b", a=16),
                                ksl.unsqueeze(-1).broadcast_to([128, 16, 16]),
                                1.0 / 32.0,
                                ksl.unsqueeze(1).broadcast_to([128, 16, 16]),
                                op0=ALU.mult,
                                op1=ALU.mult,
                            )
                            nc.gpsimd.tensor_scalar_mul(
                                phik[:, s, 256:272], ksl, 0.25)

            # ones-slot column of each head's v-ext block (after the stage-1
            # copies, which write projection zeros there)
            for h in range(HPC):
                _fill(nc, qkv_s[:, :, h * PH + 160:h * PH + 161], 1.0)

            # ---- Stage 2: chunked scan, heads interleaved per chunk, with
            # ---- the output projection (stage 3) folded in per chunk
            with (
                tc.tile_pool(name=f"u{rep}", bufs=1, space="PSUM") as up,
                tc.tile_pool(name=f"pz{rep}", bufs=3, space="PSUM") as pzp,
                tc.tile_pool(name=f"kv{rep}", bufs=1, space="PSUM") as kvp,
                tc.tile_pool(name=f"st{rep}", bufs=2) as stp,
                tc.tile_pool(name=f"ost{rep}", bufs=2) as osp,
            ):
                ksizes = (128, 128, 17)
                koffs = (0, 128, 256)
                # per-head state: [kvA(132) | kvB(132) | kvC rows 0:17 (132)]
                kvm_h = [kvp.tile([128, 396], F32, name=f"kvm{h}")
                         for h in range(HPC)]
                kvt_h = [(kvm_h[h][:, 0:132], kvm_h[h][:, 132:264],
                          kvm_h[h][0:17, 264:396]) for h in range(HPC)]
                # score PSUM, manual ping-pong pair (2 banks)
                put = up.tile([128, 2, 384], F32, name="put")
                snap_h = [None] * HPC
                pu_h = [None] * HPC
                stm_h = [None] * HPC
                for n in range(NCH):
                    cs = slice(n * CH, (n + 1) * CH)
                    cs2 = slice(n * CH + 128, (n + 1) * CH)
                    ms0 = slice(n * CH, n * CH + 128)
                    ms1 = slice(n * CH + 128, (n + 1) * CH)
                    for h in range(HPC):
                        # u[m, c] = k_m . q_c ; layout [m0 x (c0|c1) | m1 x c1]
                        pu = put[:, (n * HPC + h) % 2, :]
                        nc.tensor.matmul(
                            pu[:, 0:256], kt_h[h][:, ms0], qot_h[h][0:16, cs],
                            start=True, stop=True, skip_group_check=True,
                        )
                        nc.tensor.matmul(
                            pu[:, 256:384], kt_h[h][:, ms1], qot_h[h][0:16, cs2],
                            start=True, stop=True, skip_group_check=True,
                        )
                        # st = (u/sqrt(32) + 1/sqrt(2))^2 + 0.5, causal-masked
                        straw = stp.tile([128, 384], F32, tag="straw")
                        nc.scalar.activation(
                            straw[:], pu[:], AF.Square,
                            bias=sqb_s[:], scale=_SQ_SCALE,
                        )
                        stm = stp.tile([128, 384], BF16, tag="stm",
                                       name=f"stm{rep}_{h}_{n}")
                        nc.vector.scalar_tensor_tensor(
                            stm[:], straw[:], 0.5, mask_s[:],
                            op0=ALU.add, op1=ALU.mult,
                        )
                        pu_h[h] = pu
                        stm_h[h] = stm
                    for h in range(HPC):
                        voff = h * PH + 2 * FD
                        phiqt = phiqt_h[h]
                        phik = phik_h[h]
                        stm = stm_h[h]
                        snap = snap_h[h]
                        # merged [po (cols 0:256) | z row (cols 256:512)]
                        pz = pzp.tile([128, 512], F32, tag="pz",
                                      name=f"pz{rep}_{h}_{n}")
                        po = pz[:, 0:256]
                        zz = pz[0:1, 256:512]
                        nmm = 2 if n == 0 else 5
                        # z[c] = sum_m st[m,c] + phiQ[c] . ks
                        nc.tensor.matmul(
                            zz[0:1, 0:256], ones_s[:], stm[:, 0:256],
                            start=True, stop=(nmm == 2),
                        )
                        nc.tensor.matmul(
                            zz[0:1, 128:256], ones_s[:], stm[:, 256:384],
                            start=False, stop=(n == 0), skip_group_check=True,
                        )
                        if n > 0:
                            for t in range(3):
                                kd = ksizes[t]
                                rhs = (phiqt[0:128, t, cs] if t < 2
                                       else qot_h[h][0:17, cs])
                                nc.tensor.matmul(
                                    zz[0:1, 0:256], snap[t][0:kd, 128:129], rhs,
                                    start=False, stop=(t == 2),
                                    skip_group_check=True,
                                )
                        # 1/z (fp32 fast approx), broadcast to partitions
                        zr = stp.tile([1, CH], F32, tag="zr")
                        nc.vector.reciprocal_approx_fast(zr[:], zz[0:1, :])
                        zrb = stp.tile([128, CH], F32, tag="zrb",
                                       name=f"zrb{rep}_{h}_{n}")
                        nc.gpsimd.partition_broadcast(zrb[:], zr[0:1, :])
                        # o^T[d, c] = sum_m v[m,d] st[m,c] + sum_D kv[D,d] phiQ^T[D,c]
                        oi = 2
                        nc.tensor.matmul(
                            po[:, 0:256], qkv_s[:, 2 * n, voff:voff + 128],
                            stm[:, 0:256], start=True, stop=(nmm == 2),
                        )
                        nc.tensor.matmul(
                            po[:, 128:256], qkv_s[:, 2 * n + 1, voff:voff + 128],
                            stm[:, 256:384], start=False, stop=(oi == nmm),
                            skip_group_check=True,
                        )
                        if n > 0:
                            for t in range(3):
                                kd = ksizes[t]
                                rhs = (phiqt[0:128, t, cs] if t < 2
                                       else qot_h[h][0:17, cs])
                                oi += 1
                                nc.tensor.matmul(
                                    po[:, 0:256], snap[t][0:kd, 0:128], rhs,
                                    start=False, stop=(oi == nmm),
                                    skip_group_check=True,
                                )
                        # normalized, transposed output slice
                        nc.vector.tensor_tensor(
                            onorm[:, h, cs], po[:, :], zrb[:], op=ALU.mult,
                        )
                        # state += phiK_chunk^T @ [v | 1]
                        kvt = kvt_h[h]
                        for mt in range(2):
                            s = 2 * n + mt
                            for t in range(3):
                                kd = ksizes[t]
                                co = koffs[t]
                                nc.tensor.matmul(
                                    kvt[t][0:kd, 0:129],
                                    phik[:, s, co:co + kd],
                                    qkv_s[:, s, voff:voff + 129],
                                    start=(n == 0 and mt == 0),
                                    stop=(n == NCH - 1 and mt == 1),
                                )
                        if n < NCH - 1:
                            sA = stp.tile([128, 132], BF16, tag=f"snapA{h}",
                                          name=f"snA{rep}_{h}_{n}")
                            sB = stp.tile([128, 132], BF16, tag=f"snapB{h}",
                                          name=f"snB{rep}_{h}_{n}")
                            sC = stp.tile([17, 132], BF16, tag=f"snapC{h}",
                                          name=f"snC{rep}_{h}_{n}")
                            nc.vector.tensor_copy(sA[:, 0:129], kvt[0][:, 0:129])
                            nc.scalar.copy(sB[:, 0:129], kvt[1][:, 0:129])
                            nc.scalar.copy(sC[0:17, 0:129], kvt[2][0:17, 0:129])
                            snap_h[h] = (sA, sB, sC)
                    # output projection for this chunk's two seq tiles
                    for s in (2 * n, 2 * n + 1):
                        sl = slice(s * 128, (s + 1) * 128)
                        ob = osp.tile([128, H], BF16, tag="ob",
                                      name=f"ob{rep}_{s}")
                        for j in range(3):
                            pso = pzp.tile([128, 512], F32, tag="pz",
                                           name=f"pso{rep}_{s}_{j}")
                            for h in range(HPC):
                                nc.tensor.matmul(
                                    pso[:],
                                    onorm[:, h, sl],
                                    wo_s[:, h, j * 512:(j + 1) * 512],
                                    start=(h == 0),
                                    stop=(h == HPC - 1),
                                )
                            if j == 0:
                                nc.vector.tensor_copy(
                                    ob[:, j * 512:(j + 1) * 512], pso[:])
                            else:
                                nc.scalar.copy(
                                    ob[:, j * 512:(j + 1) * 512], pso[:])
                        eng = nc.sync if s % 2 == 0 else nc.scalar
                        eng.dma_start(out[sl, :], ob[:])
            ctx_r.close()

    nc.compile()
    return nc


_NC_CACHE = None


def _get_nc():
    global _NC_CACHE
    if _NC_CACHE is None:
        _NC_CACHE = _build_nc()
    return _NC_CACHE


def _in_maps(hidden_states, Wq, Wk, Wv, Wo, reps=1):
    ut = (np.arange(128)[:, None] <= np.arange(128)[None, :]).astype(np.float32)
    maskt = np.concatenate([ut, np.ones((128, 128), np.float32), ut], axis=1)
    ident = np.eye(128, dtype=np.float32)
    maps = []
    for c in range(8):
        b, hg = c // 4, c % 4
        heads = [hg * HPC + j for j in range(HPC)]
        xtb = np.ascontiguousarray(hidden_states[b].T)
        wqkv = np.zeros((H, W3), np.float32)
        wqkt = np.zeros((H, WT), np.float32)
        wot = np.empty((HPC * HD, H), np.float32)
        for j, hh in enumerate(heads):
            o = j * PH
            wqkv[:, o:o + FD] = Wq[hh * FD:(hh + 1) * FD].T
            wqkv[:, o + FD:o + 2 * FD] = Wk[hh * FD:(hh + 1) * FD].T
            wqkv[:, o + 2 * FD:o + 2 * FD + HD] = Wv[hh * HD:(hh + 1) * HD].T
            # o + 160 is the v-ext ones-slot (zero weights)
            wqkt[:, 32 * j:32 * j + 16] = Wk[hh * FD:(hh + 1) * FD].T
            wqkt[:, 32 * j + 16:32 * j + 32] = Wq[hh * FD:(hh + 1) * FD].T
            wot[j * HD:(j + 1) * HD, :] = Wo[:, hh * HD:(hh + 1) * HD].T
        maps.append({
            "xt": xtb.astype(BF_NP),
            "wqkv": wqkv.astype(BF_NP),
            "wqkt": wqkt.astype(BF_NP),
            "wot": wot.astype(BF_NP),
            "maskt": maskt.astype(BF_NP),
            "ident": ident.astype(BF_NP),
            "onesrow": np.ones((1, L), BF_NP),
            "cachebust": np.zeros(_bust_shape(reps), np.float32),
        })
    return maps


def kernel(hidden_states, Wq, Wk, Wv, Wo):
    nc = _get_nc()
    maps = _in_maps(
        np.asarray(hidden_states, np.float32), np.asarray(Wq, np.float32),
        np.asarray(Wk, np.float32), np.asarray(Wv, np.float32),
        np.asarray(Wo, np.float32),
    )
    res = run_bass_kernel_spmd(nc, maps, core_ids=list(range(8)))
    out = np.zeros((B, L, H), np.float32)
    for c in range(8):
        out[c // 4] += res.results[c]["out"].astype(np.float32)
    return out


# revision 25
# speedup vs baseline: 1.4502x; 1.1597x over previous
"""Trainium2 Bass kernel for BasedLinearAttention (Taylor-feature linear attention).

Full inputs -> full output. Sharding: data-parallel over batch (2) x
tensor-parallel over heads (12 heads / 4 cores = 3 heads/core); 8 cores total.
Each core computes its 3 heads' attention + the partial output projection;
the host sums the 4 per-core partials of each batch (TP row-parallel reduce).

Math notes:
  phi(x) = [1, x/d^(1/4), vec(x (x) x)/(sqrt2 sqrt d)], d=16, D=273.
  phi(k).phi(q) = 1 + u/sqrt(d) + u^2/(2d),  u = k.q
               = (u/sqrt(2d) + 1/sqrt(2))^2 + 0.5
  so intra-chunk scores never materialize phi. The result is chunk-size
  invariant; we use CHUNK=256 (reference uses 64).
  State feature order: [q(x)q (256) | x (16) | 1] with the
  s^2 = [1/(2d)=1/32, 1/sqrt(d)=1/4, 1] scaling folded into the K-side
  features (exact powers of two).
All matmuls run in bf16 (f32 PSUM accumulation): full PE rate at any free
size, no fp32-mode power throttle. Verified end-to-end max-rel ~4e-3 vs
the f32 reference (gate 2e-2).

q^T/k^T come from a transposed projection pass (W stationary, x^T moving)
instead of per-tile PE transposes; per-head tiles are carved out of the
packed PSUM result with SBUF->SBUF shift DMAs (engines can't cross
partitions, DMA can).

Intra-chunk causal structure: with chunk 256 = 2x128 seq tiles, only the
two diagonal 128x128 blocks need masking, the lower block is all-ones and
the upper block is identically zero and never computed. Scores for one
chunk live in a [128, 384] layout = [m0 x (c0|c1) | m1 x c1].
"""

import sys
import zlib
import numpy as np
import ml_dtypes
from contextlib import ExitStack

sys.path.insert(0, "/opt/trn_rl_repo")
sys.path.insert(0, "/opt/trn_rl_repo/pypackages")

import concourse.bass as bass
import concourse.tile as tile
from concourse import bacc
from concourse import mybir
from concourse.bass_utils import run_bass_kernel_spmd

F32 = mybir.dt.float32
BF16 = mybir.dt.bfloat16
ALU = mybir.AluOpType
AF = mybir.ActivationFunctionType
BF_NP = ml_dtypes.bfloat16

B, L, H = 2, 2048, 1536
NH, FD, HD = 12, 16, 128
HPC = 3            # heads per core
CH = 256           # chunk length
NCH = L // CH      # 8 chunks
NKT = H // 128     # 12 contraction tiles for projections
NST = L // 128     # 16 seq tiles
PH = 2 * FD + HD + 1   # 161: [q | k | v | ones-slot] per head
W3 = HPC * PH + 1      # 484 (padded even)
WT = HPC * 2 * FD      # 96: [k0 q0 k1 q1 k2 q2] transposed-projection cols

_SQ_SCALE = 1.0 / np.sqrt(2.0 * FD)   # 1/sqrt(32)
_SQ_BIAS = 1.0 / np.sqrt(2.0)

# The neuron NEFF cache can false-hit across different BIR with identical
# HLO I/O shapes; encode (source crc, reps) into a dummy input's shape.
try:
    _SRC_CRC = zlib.crc32(open(__file__, "rb").read()) % 1024
except OSError:
    _SRC_CRC = 0


def _bust_shape(reps):
    return [reps, 8 + _SRC_CRC]


def _fill(nc, ap, val):
    # real memset: the Identity(in*0 + bias) ACT trick reads uninitialized
    # SBUF, and 0 * NaN = NaN on a cold core. DVE, not gpsimd: any gpsimd
    # activity drops the PE utilization cap to 4/8 for its whole window.
    nc.vector.memset(ap, float(val))


def _build_nc(reps=1):
    nc = bacc.Bacc("TRN2", target_bir_lowering=False, debug=False)
    xt = nc.declare_dram_parameter("xt", [H, L], BF16, isOutput=False)
    wqkv = nc.declare_dram_parameter("wqkv", [H, W3], BF16, isOutput=False)
    wqkt = nc.declare_dram_parameter("wqkt", [H, WT], BF16, isOutput=False)
    wot = nc.declare_dram_parameter("wot", [HPC * HD, H], BF16, isOutput=False)
    maskt = nc.declare_dram_parameter("maskt", [128, 384], BF16, isOutput=False)
    ident = nc.declare_dram_parameter("ident", [128, 128], BF16, isOutput=False)
    onesrow = nc.declare_dram_parameter("onesrow", [1, L], BF16, isOutput=False)
    out = nc.declare_dram_parameter("out", [L, H], BF16, isOutput=True)
    nc.declare_dram_parameter("cachebust", _bust_shape(reps), F32, isOutput=False)

    with tile.TileContext(nc) as tc, ExitStack() as ctx:
        const = ctx.enter_context(tc.tile_pool(name="const", bufs=1))
        wqkt_s = const.tile([128, NKT, WT], BF16)
        nc.sync.dma_start(wqkt_s[:], wqkt.rearrange("(k p) n -> p k n", p=128))
        wq_s = const.tile([128, NKT, W3], BF16)
        nc.scalar.dma_start(wq_s[:], wqkv.rearrange("(k p) n -> p k n", p=128))
        xt_t = const.tile([128, NKT, L], BF16, name="xt_t")
        for quarter in range(4):
            qsl = slice(quarter * 512, (quarter + 1) * 512)
            for kt in range(NKT):
                eng = nc.sync if kt % 2 == 0 else nc.scalar
                eng.dma_start(xt_t[:, kt, qsl], xt[kt * 128:(kt + 1) * 128, qsl])
        ident_s = const.tile([128, 128], BF16)
        nc.sync.dma_start(ident_s[:], ident[:])
        mask_s = const.tile([128, 384], BF16)
        nc.sync.dma_start(mask_s[:], maskt[:])
        ones_s = const.tile([128, 1], BF16)
        _fill(nc, ones_s[:], 1.0)
        onesrow_s = const.tile([1, 128], BF16)
        _fill(nc, onesrow_s[:], 1.0)
        sqb_s = const.tile([128, 1], F32)
        nc.vector.memset(sqb_s[:], _SQ_BIAS)
        wo_s = const.tile([128, HPC, H], BF16)
        nc.scalar.dma_start(wo_s[:], wot.rearrange("(h p) n -> p h n", p=128))
        qkv_s = const.tile([128, NST, W3], BF16)
        onorm = const.tile([128, HPC, L], BF16)
        # per-head transposed tiles: k^T [16, L], [q;1]^T [17, L]
        kt_h = [const.tile([16, L], BF16, name=f"kt{h}") for h in range(HPC)]
        qot_h = [const.tile([17, L], BF16, name=f"qot{h}") for h in range(HPC)]
        for h in range(HPC):
            nc.sync.dma_start(qot_h[h][16:17, :], onesrow[:])

        for rep in range(reps):
            ctx_r = ExitStack()
            hp = ctx_r.enter_context(tc.tile_pool(name=f"hd{rep}", bufs=1))
            phiqt_h = [hp.tile([128, 2, L], BF16, name=f"phiqt{h}")
                       for h in range(HPC)]
            phik_h = [hp.tile([128, NST, 273], BF16, name=f"phik{h}")
                      for h in range(HPC)]

            # ---- Stage 1: projections fused with phi-feature building ----
            with (
                tc.tile_pool(name=f"qkvps{rep}", bufs=4, space="PSUM") as qps,
                tc.tile_pool(name=f"qktps{rep}", bufs=2, space="PSUM") as tps,
                tc.tile_pool(name=f"tp{rep}", bufs=2, space="PSUM") as tp,
                tc.tile_pool(name=f"qkstg{rep}", bufs=2) as stg,
                tc.tile_pool(name=f"p2{rep}", bufs=2) as p2,
            ):
                for h in range(HPC):
                    _fill(nc, phik_h[h][:, :, 272:273], 1.0)

                for quarter in range(4):
                    qsl = slice(quarter * 512, (quarter + 1) * 512)
                    # transposed q/k projection: out rows [k0 q0 k1 q1 k2 q2]
                    pt = tps.tile([WT, 512], F32, tag="pt")
                    for kt in range(NKT):
                        nc.tensor.matmul(
                            pt[:], wqkt_s[:, kt, :], xt_t[:, kt, qsl],
                            start=(kt == 0), stop=(kt == NKT - 1),
                        )
                    qkstg = stg.tile([WT, 512], BF16, tag="stg")
                    if quarter % 2 == 0:
                        nc.vector.tensor_copy(qkstg[:], pt[:])
                    else:
                        nc.scalar.copy(qkstg[:], pt[:])
                    for h in range(HPC):
                        nc.scalar.dma_start(
                            kt_h[h][:, qsl], qkstg[32 * h:32 * h + 16, :])
                        nc.scalar.dma_start(
                            qot_h[h][0:16, qsl], qkstg[32 * h + 16:32 * h + 32, :])
                    # seq-major q/k/v projection + phi features per seq tile
                    for s4 in range(4):
                        s = quarter * 4 + s4
                        sl = slice(s * 128, (s + 1) * 128)
                        ps = qps.tile([128, W3], F32, tag="ps")
                        for kt in range(NKT):
                            nc.tensor.matmul(
                                ps[:],
                                xt_t[:, kt, sl],
                                wq_s[:, kt, :],
                                start=(kt == 0),
                                stop=(kt == NKT - 1),
                            )
                        if s % 4 == 0:
                            nc.vector.tensor_copy(qkv_s[:, s, :], ps[:])
                        else:
                            nc.scalar.copy(qkv_s[:, s, :], ps[:])
                        for h in range(HPC):
                            qoff = h * PH
                            phiqt = phiqt_h[h]
                            phik = phik_h[h]
                            qsl_ = qkv_s[:, s, qoff:qoff + 16]
                            p2n = p2.tile([128, 256], BF16, tag="p2n")
                            nc.vector.tensor_tensor(
                                p2n[:].rearrange("p (a b) -> p a b", a=16),
                                qsl_.unsqueeze(-1).broadcast_to([128, 16, 16]),
                                qsl_.unsqueeze(1).broadcast_to([128, 16, 16]),
                                op=ALU.mult,
                            )
                            p2t = tp.tile([128, 256], BF16, tag="p2t")
                            nc.tensor.transpose(
                                p2t[:, 0:128], p2n[:, 0:128], ident_s[:])
                            nc.tensor.transpose(
                                p2t[:, 128:256], p2n[:, 128:256], ident_s[:])
                            if (s + h) % 2 == 0:
                                nc.vector.tensor_copy(
                                    phiqt[:, :, sl],
                                    p2t[:].rearrange("p (t c) -> p t c", t=2),
                                )
                            else:
                                nc.scalar.copy(
                                    phiqt[:, :, sl],
                                    p2t[:].rearrange("p (t c) -> p t c", t=2),
                                )
                            ksl = qkv_s[:, s, qoff + FD:qoff + 2 * FD]
                            nc.vector.scalar_tensor_tensor(
                                phik[:, s, 0:256].rearrange("p (a b) -> p a b", a=16),
                                ksl.unsqueeze(-1).broadcast_to([128, 16, 16]),
                                1.0 / 32.0,
                                ksl.unsqueeze(1).broadcast_to([128, 16, 16]),
                                op0=ALU.mult,
                                op1=ALU.mult,
                            )
                            nc.gpsimd.tensor_scalar_mul(
                                phik[:, s, 256:272], ksl, 0.25)

            # ones-slot column of each head's v-ext block (after the stage-1
            # copies, which write projection zeros there)
            for h in range(HPC):
                _fill(nc, qkv_s[:, :, h * PH + 160:h * PH + 161], 1.0)

            # ---- Stage 2: chunked scan, heads interleaved per chunk, with
            # ---- the output projection (stage 3) folded in per chunk
            with (
                tc.tile_pool(name=f"u{rep}", bufs=1, space="PSUM") as up,
                tc.tile_pool(name=f"pz{rep}", bufs=3, space="PSUM") as pzp,
                tc.tile_pool(name=f"kv{rep}", bufs=1, space="PSUM") as kvp,
                tc.tile_pool(name=f"st{rep}", bufs=2) as stp,
                tc.tile_pool(name=f"ost{rep}", bufs=2) as osp,
            ):
                ksizes = (128, 128, 17)
                koffs = (0, 128, 256)
                # per-head state: [kvA(132) | kvB(132) | kvC rows 0:17 (132)]
                kvm_h = [kvp.tile([128, 396], F32, name=f"kvm{h}")
                         for h in range(HPC)]
                kvt_h = [(kvm_h[h][:, 0:132], kvm_h[h][:, 132:264],
                          kvm_h[h][0:17, 264:396]) for h in range(HPC)]
                # score PSUM, manual ping-pong pair (2 banks)
                put = up.tile([128, 2, 384], F32, name="put")
                snap_h = [None] * HPC
                pu_h = [None] * HPC
                stm_h = [None] * HPC
                for n in range(NCH):
                    cs = slice(n * CH, (n + 1) * CH)
                    cs2 = slice(n * CH + 128, (n + 1) * CH)
                    ms0 = slice(n * CH, n * CH + 128)
                    ms1 = slice(n * CH + 128, (n + 1) * CH)
                    for h in range(HPC):
                        # u[m, c] = k_m . q_c ; layout [m0 x (c0|c1) | m1 x c1]
                        pu = put[:, (n * HPC + h) % 2, :]
                        nc.tensor.matmul(
                            pu[:, 0:256], kt_h[h][:, ms0], qot_h[h][0:16, cs],
                            start=True, stop=True, skip_group_check=True,
                        )
                        nc.tensor.matmul(
                            pu[:, 256:384], kt_h[h][:, ms1], qot_h[h][0:16, cs2],
                            start=True, stop=True, skip_group_check=True,
                        )
                        # st = (u/sqrt(32) + 1/sqrt(2))^2 + 0.5, causal-masked
                        straw = stp.tile([128, 384], F32, tag="straw")
                        nc.scalar.activation(
                            straw[:], pu[:], AF.Square,
                            bias=sqb_s[:], scale=_SQ_SCALE,
                        )
                        stm = stp.tile([128, 384], BF16, tag="stm",
                                       name=f"stm{rep}_{h}_{n}")
                        nc.vector.scalar_tensor_tensor(
                            stm[:], straw[:], 0.5, mask_s[:],
                            op0=ALU.add, op1=ALU.mult,
                        )
                        pu_h[h] = pu
                        stm_h[h] = stm
                    for h in range(HPC):
                        voff = h * PH + 2 * FD
                        phiqt = phiqt_h[h]
                        phik = phik_h[h]
                        stm = stm_h[h]
                        snap = snap_h[h]
                        # merged [po (cols 0:256) | z row (cols 256:512)]
                        pz = pzp.tile([128, 512], F32, tag="pz",
                                      name=f"pz{rep}_{h}_{n}")
                        po = pz[:, 0:256]
                        zz = pz[0:1, 256:512]
                        nmm = 2 if n == 0 else 5
                        # z[c] = sum_m st[m,c] + phiQ[c] . ks
                        nc.tensor.matmul(
                            zz[0:1, 0:256], ones_s[:], stm[:, 0:256],
                            start=True, stop=(nmm == 2),
                        )
                        nc.tensor.matmul(
                            zz[0:1, 128:256], ones_s[:], stm[:, 256:384],
                            start=False, stop=(n == 0), skip_group_check=True,
                        )
                        if n > 0:
                            for t in range(3):
                                kd = ksizes[t]
                                rhs = (phiqt[0:128, t, cs] if t < 2
                                       else qot_h[h][0:17, cs])
                                nc.tensor.matmul(
                                    zz[0:1, 0:256], snap[t][0:kd, 128:129], rhs,
                                    start=False, stop=(t == 2),
                                    skip_group_check=True,
                                )
                        # 1/z (fp32 fast approx), broadcast to partitions
                        zr = stp.tile([1, CH], F32, tag="zr")
                        nc.vector.reciprocal_approx_fast(zr[:], zz[0:1, :])
                        zrb = stp.tile([128, CH], F32, tag="zrb",
                                       name=f"zrb{rep}_{h}_{n}")
                        nc.gpsimd.partition_broadcast(zrb[:], zr[0:1, :])
                        # o^T[d, c] = sum_m v[m,d] st[m,c] + sum_D kv[D,d] phiQ^T[D,c]
                        oi = 2
                        nc.tensor.matmul(
                            po[:, 0:256], qkv_s[:, 2 * n, voff:voff + 128],
                            stm[:, 0:256], start=True, stop=(nmm == 2),
                        )
                        nc.tensor.matmul(
                            po[:, 128:256], qkv_s[:, 2 * n + 1, voff:voff + 128],
                            stm[:, 256:384], start=False, stop=(oi == nmm),
                            skip_group_check=True,
                        )
                        if n > 0:
                            for t in range(3):
                                kd = ksizes[t]
                                rhs = (phiqt[0:128, t, cs] if t < 2
                                       else qot_h[h][0:17, cs])
                                oi += 1
                                nc.tensor.matmul(
                                    po[:, 0:256], snap[t][0:kd, 0:128], rhs,
                                    start=False, stop=(oi == nmm),
                                    skip_group_check=True,
                                )
                        # normalized, transposed output slice
                        nc.vector.tensor_tensor(
                            onorm[:, h, cs], po[:, :], zrb[:], op=ALU.mult,
                        )
                        # state += phiK_chunk^T @ [v | 1]
                        kvt = kvt_h[h]
                        for mt in range(2):
                            s = 2 * n + mt
                            for t in range(3):
                                kd = ksizes[t]
                                co = koffs[t]
                                nc.tensor.matmul(
                                    kvt[t][0:kd, 0:129],
                                    phik[:, s, co:co + kd],
                                    qkv_s[:, s, voff:voff + 129],
                                    start=(n == 0 and mt == 0),
                                    stop=(n == NCH - 1 and mt == 1),
                                )
                        if n < NCH - 1:
                            sA = stp.tile([128, 132], BF16, tag=f"snapA{h}",
                                          name=f"snA{rep}_{h}_{n}")
                            sB = stp.tile([128, 132], BF16, tag=f"snapB{h}",
                                          name=f"snB{rep}_{h}_{n}")
                            sC = stp.tile([17, 132], BF16, tag=f"snapC{h}",
                                          name=f"snC{rep}_{h}_{n}")
                            nc.vector.tensor_copy(sA[:, 0:129], kvt[0][:, 0:129])
                            nc.scalar.copy(sB[:, 0:129], kvt[1][:, 0:129])
                            nc.scalar.copy(sC[0:17, 0:129], kvt[2][0:17, 0:129])
                            snap_h[h] = (sA, sB, sC)
                    # output projection for this chunk's two seq tiles
                    for s in (2 * n, 2 * n + 1):
                        sl = slice(s * 128, (s + 1) * 128)
                        ob = osp.tile([128, H], BF16, tag="ob",
                                      name=f"ob{rep}_{s}")
                        for j in range(3):
                            pso = pzp.tile([128, 512], F32, tag="pz",
                                           name=f"pso{rep}_{s}_{j}")
                            for h in range(HPC):
                                nc.tensor.matmul(
                                    pso[:],
                                    onorm[:, h, sl],
                                    wo_s[:, h, j * 512:(j + 1) * 512],
                                    start=(h == 0),
                                    stop=(h == HPC - 1),
                                )
                            if j == 0:
                                nc.vector.tensor_copy(
                                    ob[:, j * 512:(j + 1) * 512], pso[:])
                            else:
                                nc.scalar.copy(
                                    ob[:, j * 512:(j + 1) * 512], pso[:])
                        eng = nc.sync if s % 2 == 0 else nc.scalar
                        eng.dma_start(out[sl, :], ob[:])
            ctx_r.close()

    nc.compile()
    return nc


_NC_CACHE = None


def _get_nc():
    global _NC_CACHE
    if _NC_CACHE is None:
        _NC_CACHE = _build_nc()
    return _NC_CACHE


def _in_maps(hidden_states, Wq, Wk, Wv, Wo, reps=1):
    ut = (np.arange(128)[:, None] <= np.arange(128)[None, :]).astype(np.float32)
    maskt = np.concatenate([ut, np.ones((128, 128), np.float32), ut], axis=1)
    ident = np.eye(128, dtype=np.float32)
    maps = []
    for c in range(8):
        b, hg = c // 4, c % 4
        heads = [hg * HPC + j for j in range(HPC)]
        xtb = np.ascontiguousarray(hidden_states[b].T)
        wqkv = np.zeros((H, W3), np.float32)
        wqkt = np.zeros((H, WT), np.float32)
        wot = np.empty((HPC * HD, H), np.float32)
        for j, hh in enumerate(heads):
            o = j * PH
            wqkv[:, o:o + FD] = Wq[hh * FD:(hh + 1) * FD].T
            wqkv[:, o + FD:o + 2 * FD] = Wk[hh * FD:(hh + 1) * FD].T
            wqkv[:, o + 2 * FD:o + 2 * FD + HD] = Wv[hh * HD:(hh + 1) * HD].T
            # o + 160 is the v-ext ones-slot (zero weights)
            wqkt[:, 32 * j:32 * j + 16] = Wk[hh * FD:(hh + 1) * FD].T
            wqkt[:, 32 * j + 16:32 * j + 32] = Wq[hh * FD:(hh + 1) * FD].T
            wot[j * HD:(j + 1) * HD, :] = Wo[:, hh * HD:(hh + 1) * HD].T
        maps.append({
            "xt": xtb.astype(BF_NP),
            "wqkv": wqkv.astype(BF_NP),
            "wqkt": wqkt.astype(BF_NP),
            "wot": wot.astype(BF_NP),
            "maskt": maskt.astype(BF_NP),
            "ident": ident.astype(BF_NP),
            "onesrow": np.ones((1, L), BF_NP),
            "cachebust": np.zeros(_bust_shape(reps), np.float32),
        })
    return maps


def kernel(hidden_states, Wq, Wk, Wv, Wo):
    nc = _get_nc()
    maps = _in_maps(
        np.asarray(hidden_states, np.float32), np.asarray(Wq, np.float32),
        np.asarray(Wk, np.float32), np.asarray(Wv, np.float32),
        np.asarray(Wo, np.float32),
    )
    res = run_bass_kernel_spmd(nc, maps, core_ids=list(range(8)))
    out = np.zeros((B, L, H), np.float32)
    for c in range(8):
        out[c // 4] += res.results[c]["out"].astype(np.float32)
    return out
